# revision 36
# baseline (speedup 1.0000x reference)
"""Trainium2 Bass kernel for nn_Attention_34351148434119 (8 NeuronCores).

Reference computation (faithful quirks included):
  q_proj = hid @ Wq; q, gate = split(q_proj)     # q is DEAD code downstream
  k = hid @ Wk; v = hid @ Wv                     # [B,KV,S,D]
  v = RoPE(v)  (k is NOT roped; q roped but unused)
  scores = (k @ v^T) * sqrt(D) + mask; attn = softmax_t(scores)   # per kv head
  out = (tile_G(attn @ v) * sigmoid(gate)) @ Wo

Sharding: core = b*4 + j  (b = batch, j = rank in 4-core batch group).
Per batch, S=2048 is split into 16 blocks of 128 rows; core j owns blocks
{4k+j} (slot k) so every core has an identical causal workload (uniform
SPMD graph).

Numerics: rel-err budget is 2e-2 and empirically rel_err ~= 0.095 * sigma
of injected score noise, so the whole k/v->scores chain runs SINGLE-PASS
fp16 (measured rel err ~5e-3).

DMA queues are split so no stream serializes another: sync carries
hid + wqg + wo + output, vector carries cos/sin + wv + wk, scalar stages
the AllGather payload, gpsimd carries mask strips + the AG + gathered-v
retrieval.  v projection runs FIRST so the 0.5MB fp16 AllGather flies
while k projection + the gate matmul run.

Attention is a flat software pipeline over all (head, slot) pairs with
depth 2: pair i's softmax tail (PE transposes + attn@v) is emitted after
pair i+2's score matmuls, so the PE always has independent score work
while a pair's softmax chain runs on vector/scalar.  The per-chunk mask
add is FUSED into the max-reduce (tensor_tensor_reduce, negated, min) and
the online-softmax chunk rescale runs on the otherwise idle gpsimd engine,
as does the gate-sigmoid multiply.  attn stays UNSCALED through the PE
transposes; 1/sum enters once per slot via a transposed broadcast
multiplied into attn@v's PSUM result.
"""
import sys
import numpy as np

sys.path.insert(0, "/opt/trn_rl_repo")

B, S, HS = 2, 2048, 2048
H, KV, D = 16, 4, 128
G = H // KV
SCALING = float(D) ** 0.5
P = 128
NB = S // P            # 16 row blocks per batch
NCORES = 8
RANKS = 4              # cores per batch group
SLOTS = 4              # owned 128-row blocks per core
ROWS = SLOTS * P       # 512 rows per core
CHUNK = 512            # t-chunk = 4 t-tiles
NCHUNK = S // CHUNK    # 4
KT = HS // P           # 16 contraction tiles
NEG_THRESH = -1e8
DEPTH = 3              # attention software-pipeline depth (pairs in flight)

_CACHE = {}


def _mask_classes(mask):
    """Classify each (s-slot k, t-chunk c) 512x512 region of the SxS mask.

    0 = skip (everything <= NEG_THRESH: contributes exact 0 after softmax)
    1 = plain (all zeros: no add needed)
    2 = add  (mixed: stage values and add on-chip)
    Slot k rows across all cores = blocks 4k..4k+3 = rows [512k, 512k+512).
    """
    cls = [[0] * NCHUNK for _ in range(SLOTS)]
    for k in range(SLOTS):
        for c in range(NCHUNK):
            reg = mask[512 * k:512 * (k + 1), 512 * c:512 * (c + 1)]
            if (reg <= NEG_THRESH).all():
                cls[k][c] = 0
            elif (reg == 0).all():
                cls[k][c] = 1
            else:
                cls[k][c] = 2
    ok = True
    for k in range(SLOTS):
        comp = [c for c in range(NCHUNK) if cls[k][c] != 0]
        # computed chunks must be a prefix starting at 0
        if comp != list(range(len(comp))) or 0 not in comp:
            ok = False
    if ok:
        # {k : chunk c computed} must be a suffix of slots for each c
        for c in range(NCHUNK):
            ks = [k for k in range(SLOTS) if cls[k][c] != 0]
            if ks != list(range(SLOTS - len(ks), SLOTS)):
                ok = False
    if not ok:
        # fully dense fallback: always correct for any mask
        cls = [[2] * NCHUNK for _ in range(SLOTS)]
    return cls


def _build(classes):
    from contextlib import ExitStack

    from concourse import bacc, mybir, tile
    from concourse.masks import make_identity

    f32 = mybir.dt.float32
    f16 = mybir.dt.float16
    bf16 = mybir.dt.bfloat16
    Alu = mybir.AluOpType
    Act = mybir.ActivationFunctionType

    computed = [[c for c in range(NCHUNK) if classes[k][c] != 0] for k in range(SLOTS)]
    add_idx = {}
    for k in range(SLOTS):
        for c in range(NCHUNK):
            if classes[k][c] == 2:
                add_idx[(k, c)] = len(add_idx)
    n_add = max(len(add_idx), 1)

    # Process big slots first: their long softmax chains get the most overlap.
    slot_order = sorted(range(SLOTS), key=lambda k: -len(computed[k]))

    nc = bacc.Bacc("TRN2", target_bir_lowering=False, debug=False,
                   num_devices=NCORES)

    hid_d = nc.declare_dram_parameter("hid16", [HS, ROWS], f16, isOutput=False)
    wv_d = nc.declare_dram_parameter("wv16", [HS, KV * D], f16, isOutput=False)
    wk_d = nc.declare_dram_parameter("wk16", [HS, KV * D], f16, isOutput=False)
    wqg_d = nc.declare_dram_parameter("wqg", [HS, HS], f16, isOutput=False)
    wo_d = nc.declare_dram_parameter("wo", [HS, HS], f16, isOutput=False)
    cosT_d = nc.declare_dram_parameter("cosT", [D, ROWS], f32, isOutput=False)
    sinT_d = nc.declare_dram_parameter("sinT", [D, ROWS], f32, isOutput=False)
    mask_d = nc.declare_dram_parameter("maskst", [n_add, P, CHUNK], bf16,
                                       isOutput=False)
    out_d = nc.declare_dram_parameter("out", [ROWS, HS], f16, isOutput=True)

    rg = [[0, 1, 2, 3], [4, 5, 6, 7]]

    with tile.TileContext(nc) as tc, ExitStack() as ctx:
        sb = ctx.enter_context(tc.tile_pool(name="sb", bufs=2))
        ps = ctx.enter_context(tc.tile_pool(name="ps", bufs=8, space="PSUM"))
        dram = ctx.enter_context(tc.tile_pool(name="dram", bufs=1, space="DRAM"))

        # ---- constants ----
        id16 = sb.tile([P, P], f16, tag="c_idb")
        make_identity(nc, id16[:])
        ones16 = sb.tile([P, P], f16, tag="c_ones")
        nc.vector.memset(ones16[:], 1.0)
        cosT = sb.tile([D, ROWS], f32, tag="c_cos")
        sinT = sb.tile([D, ROWS], f32, tag="c_sin")

        # ---- v projection (single-pass fp16; hid on sync q, wv on vector q)
        hid = []
        pv = [ps.tile([P, ROWS], f32, tag="ps", name=f"pv{g}") for g in range(KV)]
        # spread the v-proj input streams over all three DMA queues so the
        # last (hh[kk], wt[kk]) pair lands as early as possible
        for kk in range(KT):
            hh = sb.tile([P, ROWS], f16, tag="hid", bufs=KT)
            if kk < 8:
                nc.sync.dma_start(hh[:], hid_d[kk * P:(kk + 1) * P, :])
            else:
                nc.gpsimd.dma_start(hh[:], hid_d[kk * P:(kk + 1) * P, :])
            hid.append(hh)
            wt = sb.tile([P, KV * D], f16, tag="wv", bufs=6, name=f"wv{kk}")
            if kk < 8:
                nc.scalar.dma_start(wt[:], wv_d[kk * P:(kk + 1) * P, :])
            else:
                nc.sync.dma_start(wt[:], wv_d[kk * P:(kk + 1) * P, :])
            if kk == 7:
                # cos/sin behind the scalar queue's wv half: RoPE needs
                # them ~30us in
                nc.scalar.dma_start(cosT[:], cosT_d[:, :])
                nc.scalar.dma_start(sinT[:], sinT_d[:, :])
            for g in range(KV):
                nc.tensor.matmul(pv[g][:], wt[:, g * P:(g + 1) * P], hh[:],
                                 start=(kk == 0), stop=(kk == KT - 1))

        # ---- RoPE v (f32), cast fp16, AG pack (d-major) ----
        # The AllGather is split in two (heads 0-1 / heads 2-3) so attention
        # on the first heads starts while the second half still transfers.
        HG = KV // 2 * P          # 256 rows per AG half
        agi = [dram.tile([HG, CHUNK], f16, name=f"agi{h}") for h in range(2)]
        ago = [dram.tile([RANKS * HG, CHUNK], f16, name=f"ago{h}")
               for h in range(2)]
        for g in range(KV):
            vr = sb.tile([P, ROWS], f32, tag="vraw", bufs=3)
            nc.scalar.copy(vr[:], pv[g][:])
            rot = sb.tile([P, ROWS], f32, tag="vrot", bufs=2)
            nc.vector.tensor_scalar_mul(rot[0:64, :], vr[64:128, :], -1.0)
            nc.vector.tensor_copy(rot[64:128, :], vr[0:64, :])
            nc.vector.tensor_mul(vr[:], vr[:], cosT[:])
            nc.vector.tensor_mul(rot[:], rot[:], sinT[:])
            nc.vector.tensor_add(vr[:], vr[:], rot[:])
            v16 = sb.tile([P, ROWS], f16, tag="v16", bufs=2)
            nc.scalar.copy(v16[:], vr[:])
            # AGI staging on the scalar queue: keeps sync/vector streams free
            nc.scalar.dma_start(agi[g // 2][(g % 2) * P:(g % 2 + 1) * P, :],
                                v16[:])
        # k weights + mask strips first on gpsimd so they don't queue behind
        # the AG (all land well before the AG payload is even staged)
        wkt = []
        for kk in range(KT):
            wt = sb.tile([P, KV * D], f16, tag="wk", bufs=KT, name=f"wk{kk}")
            nc.gpsimd.dma_start(wt[:], wk_d[kk * P:(kk + 1) * P, :])
            wkt.append(wt)
        mts = []
        for i in range(n_add):
            mt = sb.tile([P, CHUNK], bf16, tag="msk", bufs=n_add, name=f"msk{i}")
            nc.gpsimd.dma_start(mt[:], mask_d[i, :, :])
            mts.append(mt)
        # ---- gathered-v retrieval (gpsimd: the only AG-dependent stream) ----
        # vtg[g]: [128 d, (chunk c, rank r, 128)] fp16; one DMA per (g, r).
        vtg = [sb.tile([P, NCHUNK, RANKS, P], f16, tag="vtg", bufs=4,
                       name=f"vtg{g}") for g in range(KV)]
        for half in range(2):
            nc.gpsimd.collective_compute(
                "AllGather", mybir.AluOpType.bypass, replica_groups=rg,
                ins=[agi[half].opt()], outs=[ago[half].opt()])
            for g in (2 * half, 2 * half + 1):
                gl = g % 2
                for r in range(RANKS):
                    src = ago[half][HG * r + gl * P:HG * r + (gl + 1) * P, :]
                    src = src.rearrange("p (c i) -> p c i", c=NCHUNK, i=P)
                    nc.gpsimd.dma_start(vtg[g][:, :, r, :], src)
        # vrg[bi]: [128 t, 512 d(g-major)] fp16 row-major v, derived locally
        # by PE transposes of the gathered d-major tiles (column g filled a
        # couple of score-waves before head g's attn@v needs it)
        vrg = [sb.tile([P, KV * D], f16, tag="vrg", bufs=NB, name=f"vrg{i}")
               for i in range(NB)]

        # ---- k projection (runs while the AG flies; wk loads were issued on
        # the gpsimd queue ahead of the mask strips / AG) ----
        pk = [ps.tile([P, ROWS], f32, tag="ps", name=f"pk{g}") for g in range(KV)]
        for kk in range(KT):
            wt = wkt[kk]
            for g in range(KV):
                nc.tensor.matmul(pk[g][:], wt[:, g * P:(g + 1) * P], hid[kk][:],
                                 start=(kk == 0), stop=(kk == KT - 1))
        khi = []
        for g in range(KV):
            hi = sb.tile([P, ROWS], f16, tag="khi", bufs=KV)
            nc.scalar.mul(hi[:], pk[g][:], SCALING)
            khi.append(hi)

        # ---- gate matmul.  nblks 0-1 run as a solid block while the
        # AllGather flies; nblks 2-3 are deferred and interleaved into the
        # attention pair stream as PE filler for the softmax-chain stalls.
        sigT = [None] * H
        gate_wqb = {}
        for nblk in range(4):
            wqb = []
            for kk in range(KT):
                wt = sb.tile([P, CHUNK], f16, tag="wq", bufs=32,
                             name=f"wq{nblk}_{kk}")
                nc.sync.dma_start(
                    wt[:], wqg_d[kk * P:(kk + 1) * P,
                                 nblk * CHUNK:(nblk + 1) * CHUNK])
                wqb.append(wt)
            gate_wqb[nblk] = wqb

        def emit_gate_block(nblk, m):
            pg = ps.tile([P, ROWS], f32, tag="ps", name=f"pg{nblk}_{m}")
            for kk in range(KT):
                nc.tensor.matmul(pg[:], gate_wqb[nblk][kk][:, m * P:(m + 1) * P],
                                 hid[kk][:], start=(kk == 0),
                                 stop=(kk == KT - 1))
            t = sb.tile([P, ROWS], f16, tag="sg", bufs=19)
            nc.scalar.activation(t[:], pg[:], Act.Sigmoid)
            sigT[nblk * 4 + m] = t

        for nblk in range(3):
            for m in range(4):
                emit_gate_block(nblk, m)

        # ---- out-proj weights: prefetch the whole 8.4MB on the sync queue
        # (idle after wqg) so the final phase never waits on DMA ----
        wob = [[None] * KT for _ in range(4)]
        for nblk in range(4):
            for cc in range(KT):
                t = sb.tile([P, CHUNK], f16, tag="wo", bufs=16,
                            name=f"wo{nblk}_{cc}")
                nc.sync.dma_start(
                    t[:], wo_d[cc * P:(cc + 1) * P,
                               nblk * CHUNK:(nblk + 1) * CHUNK])
                wob[nblk][cc] = t

        # ---- attention: flat software pipeline over (head, slot) pairs ----
        def emit_vrg(g):
            for c in range(NCHUNK):
                tpv = ps.tile([P, RANKS * P], f16, tag="ps", name=f"tpv{g}_{c}")
                for r in range(RANKS):
                    nc.tensor.transpose(tpv[:, r * P:(r + 1) * P],
                                        vtg[g][:, c, r, :], id16[:])
                for r in range(RANKS):
                    if (c + r) % 2:
                        nc.scalar.copy(
                            vrg[RANKS * c + r][:, g * P:(g + 1) * P],
                            tpv[:, r * P:(r + 1) * P])
                    else:
                        nc.vector.tensor_copy(
                            vrg[RANKS * c + r][:, g * P:(g + 1) * P],
                            tpv[:, r * P:(r + 1) * P])

        def emit_scores(g, k):
            # stage A: score matmuls, mask adds, per-chunk (negated) maxima,
            # one exp wave per chunk with its OWN max as ACT bias
            comp = sorted(computed[k], key=lambda c: (classes[k][c] != 2, c))
            nchk = len(comp)
            attn = sb.tile([P, CHUNK * nchk], f16, tag="attn", bufs=DEPTH + 1,
                           padded_shape=[P, CHUNK * NCHUNK],
                           name=f"attn{g}_{k}")
            cm_all = sb.tile([P, NCHUNK], f32, tag="stat", bufs=16,
                             name=f"cma{g}_{k}")
            cs_all = sb.tile([P, NCHUNK], f32, tag="stat", bufs=16,
                             name=f"csa{g}_{k}")
            for ci, c in enumerate(comp):
                psc = ps.tile([P, CHUNK], f32, tag="ps", name=f"psc{g}_{k}_{ci}")
                nc.tensor.matmul(psc[:], khi[g][:, k * P:(k + 1) * P],
                                 vtg[g][:, c], start=True, stop=True)
                if classes[k][c] == 2:
                    nc.vector.tensor_add(psc[:], psc[:], mts[add_idx[(k, c)]][:])
                nc.vector.tensor_reduce(cm_all[:, ci:ci + 1], psc[:],
                                        mybir.AxisListType.X,
                                        Alu.max, negate=True)
                nc.scalar.activation(attn[:, ci * CHUNK:(ci + 1) * CHUNK],
                                     psc[:], Act.Exp,
                                     bias=cm_all[:, ci:ci + 1],
                                     accum_out=cs_all[:, ci:ci + 1])
            return [g, k, comp, nchk, attn, cm_all, cs_all, None, None]

        def emit_stats(sctx):
            # stage B: global max, batched rescale factors exp(m_c - M),
            # chunk rescales, total sum, 1/sum broadcast tile
            g, k, comp, nchk, attn, cm_all, cs_all, _, _ = sctx
            if nchk > 1:
                mneg = sb.tile([P, 1], f32, tag="stat", bufs=16, name="mneg")
                nc.vector.tensor_reduce(mneg[:], cm_all[:, 0:nchk],
                                        mybir.AxisListType.X, Alu.min)
                dls = sb.tile([P, NCHUNK], f32, tag="stat", bufs=16, name="dls")
                nc.vector.tensor_scalar(dls[:, 0:nchk], cm_all[:, 0:nchk],
                                        mneg[:], -1.0,
                                        Alu.subtract, Alu.mult)
                fcs = sb.tile([P, NCHUNK], f32, tag="stat", bufs=16, name="fcs")
                nc.scalar.activation(fcs[:, 0:nchk], dls[:, 0:nchk], Act.Exp)
                for ci in range(nchk):
                    nc.vector.tensor_scalar_mul(
                        attn[:, ci * CHUNK:(ci + 1) * CHUNK],
                        attn[:, ci * CHUNK:(ci + 1) * CHUNK],
                        fcs[:, ci:ci + 1])
                cts = sb.tile([P, NCHUNK], f32, tag="stat", bufs=16, name="cts")
                nc.vector.tensor_mul(cts[:, 0:nchk], cs_all[:, 0:nchk],
                                     fcs[:, 0:nchk])
                tot = sb.tile([P, 1], f32, tag="stat", bufs=16, name="tot")
                nc.vector.tensor_reduce(tot[:], cts[:, 0:nchk],
                                        mybir.AxisListType.X, Alu.add)
            else:
                tot = cs_all[:, 0:1]
            rinv = sb.tile([P, 1], f32, tag="stat", bufs=16)
            nc.vector.reciprocal(rinv[:], tot[:])
            bc = sb.tile([P, P], f16, tag="bc", bufs=4)
            nc.scalar.activation(bc[:], ones16[:], Act.Copy, scale=rinv[:])
            sctx[7] = bc

        def emit_tail(sctx, rb, avt):
            g, k, comp, nchk, attn, cm_all, cs_all, bc, _ = sctx
            # transpose attn -> per-slot attnT tile (4 blocks per PSUM tile,
            # one wide copyback per chunk)
            attnT = sb.tile([P, 4 * nchk * P], f16, tag="attnT", bufs=DEPTH + 1,
                            padded_shape=[P, 4 * NCHUNK * P],
                            name=f"attnT{g}_{k}")
            for ci, c in enumerate(comp):
                tpc = ps.tile([P, CHUNK], f16, tag="ps", name=f"tp{g}_{k}_{ci}")
                for i in range(4):
                    nc.tensor.transpose(
                        tpc[:, i * P:(i + 1) * P],
                        attn[:, ci * CHUNK + i * P:ci * CHUNK + (i + 1) * P],
                        id16[:])
                if ci % 2:
                    nc.scalar.copy(attnT[:, ci * CHUNK:(ci + 1) * CHUNK],
                                   tpc[:])
                else:
                    nc.vector.tensor_copy(attnT[:, ci * CHUNK:(ci + 1) * CHUNK],
                                          tpc[:])
            # 1/sum broadcast -> transposed column of rb
            tpb = ps.tile([P, P], f16, tag="ps", name=f"tpb{g}_{k}")
            nc.tensor.transpose(tpb[:], bc[:], id16[:])
            nc.vector.tensor_copy(rb[:, k * P:(k + 1) * P], tpb[:])
            # output block k: contiguous accumulation over slot k's t-blocks
            pav = ps.tile([P, P], f32, tag="ps", name=f"pav{g}_{k}")
            n_mm = 4 * nchk
            for ci, c in enumerate(comp):
                for i in range(4):
                    bi = 4 * c + i
                    pos = (ci * 4 + i) * P
                    mm = ci * 4 + i
                    nc.tensor.matmul(pav[:], vrg[bi][:, g * P:(g + 1) * P],
                                     attnT[:, pos:pos + P],
                                     start=(mm == 0), stop=(mm == n_mm - 1))
            nc.vector.tensor_tensor(avt[:, k * P:(k + 1) * P], pav[:],
                                    rb[:, k * P:(k + 1) * P], Alu.mult)

        gat = [None] * H
        rbavt = {}

        def head_tiles(g):
            if g not in rbavt:
                rbavt[g] = (
                    sb.tile([P, ROWS], f16, tag="rb", bufs=3, name=f"rb{g}"),
                    sb.tile([P, ROWS], f16, tag="avT", bufs=3, name=f"avt{g}"),
                    [0])
            return rbavt[g]

        def finish_pair(sctx):
            g = sctx[0]
            rb, avt, cnt = head_tiles(g)
            emit_tail(sctx, rb, avt)
            cnt[0] += 1
            if cnt[0] == SLOTS:
                # head complete: gate multiply on gpsimd
                for i in range(G):
                    t = sb.tile([P, ROWS], f16, tag="sg", bufs=19)
                    nc.gpsimd.tensor_mul(t[:], avt[:], sigT[4 * g + i][:])
                    gat[4 * g + i] = t

        # 3-stage pipeline: A(i) scores+exp | B(i-1) stats+rescale |
        # tail(i-2) PE transposes + attn@v.  Keeps every engine's in-order
        # stream supplied with the next pair's independent work while the
        # current pair's cross-engine softmax chain completes.
        pairs = [(g, k) for g in range(KV) for k in slot_order]
        gate_fill = [(3, m) for m in range(4)]
        emit_vrg(0)
        pend = []
        for idx, (g, k) in enumerate(pairs):
            head_tiles(g)
            pend.append(emit_scores(g, k))
            if idx < len(gate_fill):
                emit_gate_block(*gate_fill[idx])
            if len(pend) >= 2:
                emit_stats(pend[-2])
            if k == slot_order[1] and g + 1 < KV:
                emit_vrg(g + 1)
            if len(pend) > DEPTH:
                finish_pair(pend.pop(0))
        if pend:
            emit_stats(pend[-1])
        for sctx in pend:
            finish_pair(sctx)

        # ---- out projection (fp16; weights already resident) ----
        for nblk in range(4):
            for rt in range(SLOTS):
                po = ps.tile([P, CHUNK], f32, tag="ps")
                for cc in range(KT):
                    nc.tensor.matmul(po[:], gat[cc][:, rt * P:(rt + 1) * P],
                                     wob[nblk][cc][:], start=(cc == 0),
                                     stop=(cc == KT - 1))
                t = sb.tile([P, CHUNK], f16, tag="oev", bufs=2)
                if rt % 2:
                    nc.vector.tensor_copy(t[:], po[:])
                else:
                    nc.scalar.copy(t[:], po[:])
                nc.sync.dma_start(
                    out_d[rt * P:(rt + 1) * P, nblk * CHUNK:(nblk + 1) * CHUNK],
                    t[:])

    nc.compile()
    return nc


def kernel(hidden_states, cos, sin, attention_mask, Wq, Wk, Wv, Wo):
    import ml_dtypes
    from concourse.bass_utils import run_bass_kernel_spmd

    bf = ml_dtypes.bfloat16
    f16 = np.float16
    hidden_states = np.asarray(hidden_states, dtype=np.float32)
    cos = np.asarray(cos, dtype=np.float32)
    sin = np.asarray(sin, dtype=np.float32)
    mask = np.asarray(attention_mask, dtype=np.float32)[0, 0]
    Wq = np.asarray(Wq, dtype=np.float32)
    Wk = np.asarray(Wk, dtype=np.float32)
    Wv = np.asarray(Wv, dtype=np.float32)
    Wo = np.asarray(Wo, dtype=np.float32)

    classes = _mask_classes(mask)
    key = tuple(tuple(r) for r in classes)
    if key not in _CACHE:
        _CACHE[key] = _build(classes)
    nc = _CACHE[key]

    wv16 = np.ascontiguousarray(Wv.astype(f16))
    wk16 = np.ascontiguousarray(Wk.astype(f16))
    wqg = np.ascontiguousarray(Wq[:, HS:].astype(f16))
    wo16 = np.ascontiguousarray(Wo.astype(f16))

    in_maps = []
    for core in range(NCORES):
        b, j = divmod(core, RANKS)
        blocks = [RANKS * k + j for k in range(SLOTS)]
        rows = np.concatenate([np.arange(bi * P, (bi + 1) * P) for bi in blocks])
        hidT = np.ascontiguousarray(hidden_states[b][rows].T.astype(f16))
        strips = []
        for k in range(SLOTS):
            for c in range(NCHUNK):
                if classes[k][c] == 2:
                    bi = RANKS * k + j
                    strips.append(mask[bi * P:(bi + 1) * P,
                                       c * CHUNK:(c + 1) * CHUNK])
        if not strips:
            strips.append(np.zeros((P, CHUNK), np.float32))
        in_maps.append({
            "hid16": hidT,
            "wv16": wv16,
            "wk16": wk16,
            "wqg": wqg,
            "wo": wo16,
            "cosT": np.ascontiguousarray(cos[b][rows].T),
            "sinT": np.ascontiguousarray(sin[b][rows].T),
            "maskst": np.ascontiguousarray(np.stack(strips).astype(bf)),
        })

    res = run_bass_kernel_spmd(nc, in_maps, core_ids=list(range(NCORES)))

    out = np.empty((B, S, HS), np.float32)
    for core in range(NCORES):
        b, j = divmod(core, RANKS)
        o = np.asarray(res.results[core]["out"]).astype(np.float32)
        for k in range(SLOTS):
            bi = RANKS * k + j
            out[b, bi * P:(bi + 1) * P, :] = o[k * P:(k + 1) * P, :]
    return out


# revision 44
# speedup vs baseline: 1.1612x; 1.1612x over previous
"""Trainium2 Bass kernel for nn_Attention_34351148434119 (8 NeuronCores).

Reference computation (faithful quirks included):
  q_proj = hid @ Wq; q, gate = split(q_proj)     # q is DEAD code downstream
  k = hid @ Wk; v = hid @ Wv                     # [B,KV,S,D]
  v = RoPE(v)  (k is NOT roped; q roped but unused)
  scores = (k @ v^T) * sqrt(D) + mask; attn = softmax_t(scores)   # per kv head
  out = (tile_G(attn @ v) * sigmoid(gate)) @ Wo

Sharding: core = b*4 + j  (b = batch, j = rank in 4-core batch group).
Per batch, S=2048 is split into 16 blocks of 128 rows; core j owns blocks
{4k+j} (slot k) so every core has an identical causal workload (uniform
SPMD graph).

Numerics: rel-err budget is 2e-2 and empirically rel_err ~= 0.095 * sigma
of injected score noise, so the whole k/v->scores chain runs SINGLE-PASS
fp16 (measured rel err ~5e-3).

DMA queues are split so no stream serializes another: sync carries
hid + wqg + wo + output, vector carries cos/sin + wv + wk, scalar stages
the AllGather payload, gpsimd carries mask strips + the AG + gathered-v
retrieval.  v projection runs FIRST so the 0.5MB fp16 AllGather flies
while k projection + the gate matmul run.

Attention is a flat software pipeline over all (head, slot) pairs with
depth 2: pair i's softmax tail (PE transposes + attn@v) is emitted after
pair i+2's score matmuls, so the PE always has independent score work
while a pair's softmax chain runs on vector/scalar.  The per-chunk mask
add is FUSED into the max-reduce (tensor_tensor_reduce, negated, min) and
the online-softmax chunk rescale runs on the otherwise idle gpsimd engine,
as does the gate-sigmoid multiply.  attn stays UNSCALED through the PE
transposes; 1/sum enters once per slot via a transposed broadcast
multiplied into attn@v's PSUM result.
"""
import sys
import numpy as np

sys.path.insert(0, "/opt/trn_rl_repo")

B, S, HS = 2, 2048, 2048
H, KV, D = 16, 4, 128
G = H // KV
SCALING = float(D) ** 0.5
P = 128
NB = S // P            # 16 row blocks per batch
NCORES = 8
RANKS = 4              # cores per batch group
SLOTS = 4              # owned 128-row blocks per core
ROWS = SLOTS * P       # 512 rows per core
CHUNK = 512            # t-chunk = 4 t-tiles
NCHUNK = S // CHUNK    # 4
KT = HS // P           # 16 contraction tiles
NEG_THRESH = -1e8
DEPTH = 2              # attention software-pipeline depth (pairs in flight)

_CACHE = {}


def _mask_classes(mask):
    """Classify each (s-slot k, t-chunk c) 512x512 region of the SxS mask.

    0 = skip (everything <= NEG_THRESH: contributes exact 0 after softmax)
    1 = plain (all zeros: no add needed)
    2 = add  (mixed: stage values and add on-chip)
    Slot k rows across all cores = blocks 4k..4k+3 = rows [512k, 512k+512).
    """
    cls = [[0] * NCHUNK for _ in range(SLOTS)]
    for k in range(SLOTS):
        for c in range(NCHUNK):
            reg = mask[512 * k:512 * (k + 1), 512 * c:512 * (c + 1)]
            if (reg <= NEG_THRESH).all():
                cls[k][c] = 0
            elif (reg == 0).all():
                cls[k][c] = 1
            else:
                cls[k][c] = 2
    ok = True
    for k in range(SLOTS):
        comp = [c for c in range(NCHUNK) if cls[k][c] != 0]
        # computed chunks must be a prefix starting at 0
        if comp != list(range(len(comp))) or 0 not in comp:
            ok = False
    if ok:
        # {k : chunk c computed} must be a suffix of slots for each c
        for c in range(NCHUNK):
            ks = [k for k in range(SLOTS) if cls[k][c] != 0]
            if ks != list(range(SLOTS - len(ks), SLOTS)):
                ok = False
    if not ok:
        # fully dense fallback: always correct for any mask
        cls = [[2] * NCHUNK for _ in range(SLOTS)]
    return cls


def _build(classes):
    from contextlib import ExitStack

    from concourse import bacc, mybir, tile
    from concourse.masks import make_identity

    f32 = mybir.dt.float32
    f16 = mybir.dt.float16
    bf16 = mybir.dt.bfloat16
    f8 = mybir.dt.float8e4
    Alu = mybir.AluOpType
    Act = mybir.ActivationFunctionType

    computed = [[c for c in range(NCHUNK) if classes[k][c] != 0] for k in range(SLOTS)]
    add_idx = {}
    for k in range(SLOTS):
        for c in range(NCHUNK):
            if classes[k][c] == 2:
                add_idx[(k, c)] = len(add_idx)
    n_add = max(len(add_idx), 1)

    # Process big slots first: their long softmax chains get the most overlap.
    slot_order = sorted(range(SLOTS), key=lambda k: -len(computed[k]))

    nc = bacc.Bacc("TRN2", target_bir_lowering=False, debug=False,
                   num_devices=NCORES)

    hid_d = nc.declare_dram_parameter("hid16", [HS, ROWS], f16, isOutput=False)
    wv_d = nc.declare_dram_parameter("wv16", [HS, KV * D], f16, isOutput=False)
    wk_d = nc.declare_dram_parameter("wk16", [HS, KV * D], f16, isOutput=False)
    wqg_d = nc.declare_dram_parameter("wqg8", [HS, HS], f8, isOutput=False)
    hid8_d = nc.declare_dram_parameter("hid8", [HS, ROWS], f8, isOutput=False)
    wo_d = nc.declare_dram_parameter("wo", [HS, HS], f16, isOutput=False)
    cosT_d = nc.declare_dram_parameter("cosT", [D, ROWS], f32, isOutput=False)
    sinT_d = nc.declare_dram_parameter("sinT", [D, ROWS], f32, isOutput=False)
    mask_d = nc.declare_dram_parameter("maskst", [n_add, P, CHUNK], bf16,
                                       isOutput=False)
    out_d = nc.declare_dram_parameter("out", [ROWS, HS], f16, isOutput=True)

    rg = [[0, 1, 2, 3], [4, 5, 6, 7]]

    with tile.TileContext(nc) as tc, ExitStack() as ctx:
        sb = ctx.enter_context(tc.tile_pool(name="sb", bufs=2))
        ps = ctx.enter_context(tc.tile_pool(name="ps", bufs=8, space="PSUM"))
        dram = ctx.enter_context(tc.tile_pool(name="dram", bufs=1, space="DRAM"))

        # ---- constants ----
        id16 = sb.tile([P, P], f16, tag="c_idb")
        make_identity(nc, id16[:])
        ones16 = sb.tile([P, P], f16, tag="c_ones")
        nc.vector.memset(ones16[:], 1.0)
        cosT = sb.tile([D, ROWS], f32, tag="c_cos")
        sinT = sb.tile([D, ROWS], f32, tag="c_sin")

        # ---- v projection (single-pass fp16; hid on sync q, wv on vector q)
        hid = []
        pv = [ps.tile([P, ROWS], f32, tag="ps", name=f"pv{g}") for g in range(KV)]
        # spread the v-proj input streams over all three DMA queues so the
        # last (hh[kk], wt[kk]) pair lands as early as possible
        for kk in range(KT):
            hh = sb.tile([P, ROWS], f16, tag="hid", bufs=KT)
            if kk < 8:
                nc.sync.dma_start(hh[:], hid_d[kk * P:(kk + 1) * P, :])
            else:
                nc.gpsimd.dma_start(hh[:], hid_d[kk * P:(kk + 1) * P, :])
            hid.append(hh)
            wt = sb.tile([P, KV * D], f16, tag="wv", bufs=6, name=f"wv{kk}")
            if kk < 8:
                nc.scalar.dma_start(wt[:], wv_d[kk * P:(kk + 1) * P, :])
            else:
                nc.sync.dma_start(wt[:], wv_d[kk * P:(kk + 1) * P, :])
            if kk == 7:
                # cos/sin behind the scalar queue's wv half: RoPE needs
                # them ~30us in
                nc.scalar.dma_start(cosT[:], cosT_d[:, :])
                nc.scalar.dma_start(sinT[:], sinT_d[:, :])
            for g in range(KV):
                nc.tensor.matmul(pv[g][:], wt[:, g * P:(g + 1) * P], hh[:],
                                 start=(kk == 0), stop=(kk == KT - 1))

        # ---- RoPE v (f32), cast fp16, AG pack (d-major) ----
        # The AllGather is split in two (heads 0-1 / heads 2-3) so attention
        # on the first heads starts while the second half still transfers.
        HG = KV // 2 * P          # 256 rows per AG half
        agi = [dram.tile([HG, CHUNK], f16, name=f"agi{h}") for h in range(2)]
        ago = [dram.tile([RANKS * HG, CHUNK], f16, name=f"ago{h}")
               for h in range(2)]
        for g in range(KV):
            vr = sb.tile([P, ROWS], f32, tag="vraw", bufs=3)
            nc.scalar.copy(vr[:], pv[g][:])
            rot = sb.tile([P, ROWS], f32, tag="vrot", bufs=2)
            nc.vector.tensor_scalar_mul(rot[0:64, :], vr[64:128, :], -1.0)
            nc.vector.tensor_copy(rot[64:128, :], vr[0:64, :])
            nc.vector.tensor_mul(vr[:], vr[:], cosT[:])
            nc.vector.tensor_mul(rot[:], rot[:], sinT[:])
            nc.vector.tensor_add(vr[:], vr[:], rot[:])
            v16 = sb.tile([P, ROWS], f16, tag="v16", bufs=2)
            nc.scalar.copy(v16[:], vr[:])
            # AGI staging on the scalar queue: keeps sync/vector streams free
            nc.scalar.dma_start(agi[g // 2][(g % 2) * P:(g % 2 + 1) * P, :],
                                v16[:])
        # k weights + fp8 hidden + mask strips first on gpsimd so they don't
        # queue behind the AG (all land well before the AG payload is staged)
        wkt = []
        for kk in range(KT):
            wt = sb.tile([P, KV * D], f16, tag="wk", bufs=KT, name=f"wk{kk}")
            nc.gpsimd.dma_start(wt[:], wk_d[kk * P:(kk + 1) * P, :])
            wkt.append(wt)
        # hid8 packed for DoubleRow: tile [128, 2, ROWS], (p, i) holds
        # contraction row 256*k2 + 128*i + p
        hid8t = []
        for k2 in range(KT // 2):
            t = sb.tile([P, 2, ROWS], f8, tag="hid8", bufs=KT // 2,
                        name=f"hid8_{k2}")
            src = hid8_d[k2 * 2 * P:(k2 + 1) * 2 * P, :]
            src = src.rearrange("(i p) r -> p i r", i=2, p=P)
            nc.gpsimd.dma_start(t[:], src)
            hid8t.append(t)
        mts = []
        for i in range(n_add):
            mt = sb.tile([P, CHUNK], bf16, tag="msk", bufs=n_add, name=f"msk{i}")
            nc.gpsimd.dma_start(mt[:], mask_d[i, :, :])
            mts.append(mt)
        # ---- gathered-v retrieval (gpsimd: the only AG-dependent stream) ----
        # vtg[g]: [128 d, (chunk c, rank r, 128)] fp16; one DMA per (g, r).
        vtg = [sb.tile([P, NCHUNK, RANKS, P], f16, tag="vtg", bufs=4,
                       name=f"vtg{g}") for g in range(KV)]
        for half in range(2):
            nc.gpsimd.collective_compute(
                "AllGather", mybir.AluOpType.bypass, replica_groups=rg,
                ins=[agi[half].opt()], outs=[ago[half].opt()])
            for g in (2 * half, 2 * half + 1):
                gl = g % 2
                for r in range(RANKS):
                    src = ago[half][HG * r + gl * P:HG * r + (gl + 1) * P, :]
                    src = src.rearrange("p (c i) -> p c i", c=NCHUNK, i=P)
                    nc.gpsimd.dma_start(vtg[g][:, :, r, :], src)
        # vrg[bi]: [128 t, 512 d(g-major)] fp16 row-major v, derived locally
        # by PE transposes of the gathered d-major tiles (column g filled a
        # couple of score-waves before head g's attn@v needs it)
        vrg = [sb.tile([P, KV * D], f16, tag="vrg", bufs=NB, name=f"vrg{i}")
               for i in range(NB)]

        # ---- k projection (runs while the AG flies; wk loads were issued on
        # the gpsimd queue ahead of the mask strips / AG) ----
        pk = [ps.tile([P, ROWS], f32, tag="ps", name=f"pk{g}") for g in range(KV)]
        for kk in range(KT):
            wt = wkt[kk]
            for g in range(KV):
                nc.tensor.matmul(pk[g][:], wt[:, g * P:(g + 1) * P], hid[kk][:],
                                 start=(kk == 0), stop=(kk == KT - 1))
        khi = []
        for g in range(KV):
            hi = sb.tile([P, ROWS], f16, tag="khi", bufs=KV)
            nc.scalar.mul(hi[:], pk[g][:], SCALING)
            khi.append(hi)

        # ---- gate matmul in fp8 DoubleRow (256-deep contraction per
        # instruction, 2x PE rate; sigmoid tolerates the e4m3 noise).
        # nblks 0-2 run as a solid block while the AllGather flies; nblk 3
        # is interleaved into the attention pair stream as PE filler.
        sigT = [None] * H
        gate_wqb = {}
        for nblk in range(4):
            wqb = []
            for k2 in range(KT // 2):
                wt = sb.tile([P, 2, CHUNK], f8, tag="wq", bufs=24,
                             name=f"wq{nblk}_{k2}")
                src = wqg_d[k2 * 2 * P:(k2 + 1) * 2 * P,
                            nblk * CHUNK:(nblk + 1) * CHUNK]
                src = src.rearrange("(i p) c -> p i c", i=2, p=P)
                nc.sync.dma_start(wt[:], src)
                wqb.append(wt)
            gate_wqb[nblk] = wqb

        def emit_gate_block(nblk, m):
            pg = ps.tile([P, ROWS], f32, tag="ps", name=f"pg{nblk}_{m}")
            for k2 in range(KT // 2):
                nc.tensor.matmul(pg[:],
                                 gate_wqb[nblk][k2][:, :, m * P:(m + 1) * P],
                                 hid8t[k2][:], start=(k2 == 0),
                                 stop=(k2 == KT // 2 - 1),
                                 perf_mode=mybir.MatmulPerfMode.DoubleRow)
            t = sb.tile([P, ROWS], f16, tag="sg", bufs=19)
            nc.scalar.activation(t[:], pg[:], Act.Sigmoid)
            sigT[nblk * 4 + m] = t

        for nblk in range(3):
            for m in range(4):
                emit_gate_block(nblk, m)

        # ---- out-proj weights: prefetch the whole 8.4MB on the sync queue
        # (idle after wqg) so the final phase never waits on DMA ----
        wob = [[None] * KT for _ in range(4)]
        for nblk in range(4):
            for cc in range(KT):
                t = sb.tile([P, CHUNK], f16, tag="wo", bufs=24,
                            name=f"wo{nblk}_{cc}")
                nc.sync.dma_start(
                    t[:], wo_d[cc * P:(cc + 1) * P,
                               nblk * CHUNK:(nblk + 1) * CHUNK])
                wob[nblk][cc] = t

        # ---- attention: flat software pipeline over (head, slot) pairs ----
        def emit_vrg(g):
            for c in range(NCHUNK):
                tpv = ps.tile([P, RANKS * P], f16, tag="ps", name=f"tpv{g}_{c}")
                for r in range(RANKS):
                    nc.tensor.transpose(tpv[:, r * P:(r + 1) * P],
                                        vtg[g][:, c, r, :], id16[:])
                for r in range(RANKS):
                    if (c + r) % 2:
                        nc.scalar.copy(
                            vrg[RANKS * c + r][:, g * P:(g + 1) * P],
                            tpv[:, r * P:(r + 1) * P])
                    else:
                        nc.vector.tensor_copy(
                            vrg[RANKS * c + r][:, g * P:(g + 1) * P],
                            tpv[:, r * P:(r + 1) * P])

        def emit_scores(g, k):
            # stage A: score matmuls, mask adds, per-chunk (negated) maxima,
            # one exp wave per chunk with its OWN max as ACT bias
            comp = sorted(computed[k], key=lambda c: (classes[k][c] != 2, c))
            nchk = len(comp)
            attn = sb.tile([P, CHUNK * nchk], f16, tag="attn", bufs=DEPTH + 1,
                           padded_shape=[P, CHUNK * NCHUNK],
                           name=f"attn{g}_{k}")
            cm_all = sb.tile([P, NCHUNK], f32, tag="stat", bufs=16,
                             name=f"cma{g}_{k}")
            cs_all = sb.tile([P, NCHUNK], f32, tag="stat", bufs=16,
                             name=f"csa{g}_{k}")
            for ci, c in enumerate(comp):
                psc = ps.tile([P, CHUNK], f32, tag="ps", name=f"psc{g}_{k}_{ci}")
                nc.tensor.matmul(psc[:], khi[g][:, k * P:(k + 1) * P],
                                 vtg[g][:, c], start=True, stop=True)
                if classes[k][c] == 2:
                    nc.vector.tensor_add(psc[:], psc[:], mts[add_idx[(k, c)]][:])
                nc.vector.tensor_reduce(cm_all[:, ci:ci + 1], psc[:],
                                        mybir.AxisListType.X,
                                        Alu.max, negate=True)
                nc.scalar.activation(attn[:, ci * CHUNK:(ci + 1) * CHUNK],
                                     psc[:], Act.Exp,
                                     bias=cm_all[:, ci:ci + 1],
                                     accum_out=cs_all[:, ci:ci + 1])
            return [g, k, comp, nchk, attn, cm_all, cs_all, None, None]

        def emit_stats(sctx):
            # stage B: global max, batched rescale factors exp(m_c - M),
            # chunk rescales, total sum, 1/sum broadcast tile
            g, k, comp, nchk, attn, cm_all, cs_all, _, _ = sctx
            if nchk > 1:
                mneg = sb.tile([P, 1], f32, tag="stat", bufs=16, name="mneg")
                nc.vector.tensor_reduce(mneg[:], cm_all[:, 0:nchk],
                                        mybir.AxisListType.X, Alu.min)
                dls = sb.tile([P, NCHUNK], f32, tag="stat", bufs=16, name="dls")
                nc.vector.tensor_scalar(dls[:, 0:nchk], cm_all[:, 0:nchk],
                                        mneg[:], -1.0,
                                        Alu.subtract, Alu.mult)
                fcs = sb.tile([P, NCHUNK], f32, tag="stat", bufs=16, name="fcs")
                nc.scalar.activation(fcs[:, 0:nchk], dls[:, 0:nchk], Act.Exp)
                for ci in range(nchk):
                    nc.vector.tensor_scalar_mul(
                        attn[:, ci * CHUNK:(ci + 1) * CHUNK],
                        attn[:, ci * CHUNK:(ci + 1) * CHUNK],
                        fcs[:, ci:ci + 1])
                cts = sb.tile([P, NCHUNK], f32, tag="stat", bufs=16, name="cts")
                nc.vector.tensor_mul(cts[:, 0:nchk], cs_all[:, 0:nchk],
                                     fcs[:, 0:nchk])
                tot = sb.tile([P, 1], f32, tag="stat", bufs=16, name="tot")
                nc.vector.tensor_reduce(tot[:], cts[:, 0:nchk],
                                        mybir.AxisListType.X, Alu.add)
            else:
                tot = cs_all[:, 0:1]
            rinv = sb.tile([P, 1], f32, tag="stat", bufs=16)
            nc.vector.reciprocal(rinv[:], tot[:])
            bc = sb.tile([P, P], f16, tag="bc", bufs=4)
            nc.scalar.activation(bc[:], ones16[:], Act.Copy, scale=rinv[:])
            sctx[7] = bc

        def emit_tail(sctx, rb, avt):
            g, k, comp, nchk, attn, cm_all, cs_all, bc, _ = sctx
            # transpose attn -> per-slot attnT tile (4 blocks per PSUM tile,
            # one wide copyback per chunk)
            attnT = sb.tile([P, 4 * nchk * P], f16, tag="attnT", bufs=DEPTH + 1,
                            padded_shape=[P, 4 * NCHUNK * P],
                            name=f"attnT{g}_{k}")
            for ci, c in enumerate(comp):
                tpc = ps.tile([P, CHUNK], f16, tag="ps", name=f"tp{g}_{k}_{ci}")
                for i in range(4):
                    nc.tensor.transpose(
                        tpc[:, i * P:(i + 1) * P],
                        attn[:, ci * CHUNK + i * P:ci * CHUNK + (i + 1) * P],
                        id16[:])
                if ci % 2:
                    nc.scalar.copy(attnT[:, ci * CHUNK:(ci + 1) * CHUNK],
                                   tpc[:])
                else:
                    nc.vector.tensor_copy(attnT[:, ci * CHUNK:(ci + 1) * CHUNK],
                                          tpc[:])
            # 1/sum broadcast -> transposed column of rb
            tpb = ps.tile([P, P], f16, tag="ps", name=f"tpb{g}_{k}")
            nc.tensor.transpose(tpb[:], bc[:], id16[:])
            nc.vector.tensor_copy(rb[:, k * P:(k + 1) * P], tpb[:])
            # output block k: contiguous accumulation over slot k's t-blocks
            pav = ps.tile([P, P], f32, tag="ps", name=f"pav{g}_{k}")
            n_mm = 4 * nchk
            for ci, c in enumerate(comp):
                for i in range(4):
                    bi = 4 * c + i
                    pos = (ci * 4 + i) * P
                    mm = ci * 4 + i
                    nc.tensor.matmul(pav[:], vrg[bi][:, g * P:(g + 1) * P],
                                     attnT[:, pos:pos + P],
                                     start=(mm == 0), stop=(mm == n_mm - 1))
            nc.vector.tensor_tensor(avt[:, k * P:(k + 1) * P], pav[:],
                                    rb[:, k * P:(k + 1) * P], Alu.mult)

        gat = [None] * H
        rbavt = {}

        def head_tiles(g):
            if g not in rbavt:
                rbavt[g] = (
                    sb.tile([P, ROWS], f16, tag="rb", bufs=3, name=f"rb{g}"),
                    sb.tile([P, ROWS], f16, tag="avT", bufs=3, name=f"avt{g}"),
                    [0])
            return rbavt[g]

        def finish_pair(sctx):
            g = sctx[0]
            rb, avt, cnt = head_tiles(g)
            emit_tail(sctx, rb, avt)
            cnt[0] += 1
            if cnt[0] == SLOTS:
                # head complete: gate multiply on gpsimd
                for i in range(G):
                    t = sb.tile([P, ROWS], f16, tag="sg", bufs=19)
                    nc.gpsimd.tensor_mul(t[:], avt[:], sigT[4 * g + i][:])
                    gat[4 * g + i] = t

        # 3-stage pipeline: A(i) scores+exp | B(i-1) stats+rescale |
        # tail(i-2) PE transposes + attn@v.  Keeps every engine's in-order
        # stream supplied with the next pair's independent work while the
        # current pair's cross-engine softmax chain completes.
        pairs = [(g, k) for g in range(KV) for k in slot_order]
        gate_fill = [(3, m) for m in range(4)]
        emit_vrg(0)
        pend = []
        for idx, (g, k) in enumerate(pairs):
            head_tiles(g)
            pend.append(emit_scores(g, k))
            if idx < len(gate_fill):
                emit_gate_block(*gate_fill[idx])
            if len(pend) >= 2:
                emit_stats(pend[-2])
            if k == slot_order[1] and g + 1 < KV:
                emit_vrg(g + 1)
            if len(pend) > DEPTH:
                finish_pair(pend.pop(0))
        if pend:
            emit_stats(pend[-1])
        for sctx in pend:
            finish_pair(sctx)

        # ---- out projection (fp16; weights already resident) ----
        for nblk in range(4):
            for rt in range(SLOTS):
                po = ps.tile([P, CHUNK], f32, tag="ps")
                for cc in range(KT):
                    nc.tensor.matmul(po[:], gat[cc][:, rt * P:(rt + 1) * P],
                                     wob[nblk][cc][:], start=(cc == 0),
                                     stop=(cc == KT - 1))
                t = sb.tile([P, CHUNK], f16, tag="oev", bufs=2)
                if rt % 2:
                    nc.vector.tensor_copy(t[:], po[:])
                else:
                    nc.scalar.copy(t[:], po[:])
                nc.sync.dma_start(
                    out_d[rt * P:(rt + 1) * P, nblk * CHUNK:(nblk + 1) * CHUNK],
                    t[:])

    nc.compile()
    return nc


def kernel(hidden_states, cos, sin, attention_mask, Wq, Wk, Wv, Wo):
    import ml_dtypes
    from concourse.bass_utils import run_bass_kernel_spmd

    bf = ml_dtypes.bfloat16
    f16 = np.float16
    hidden_states = np.asarray(hidden_states, dtype=np.float32)
    cos = np.asarray(cos, dtype=np.float32)
    sin = np.asarray(sin, dtype=np.float32)
    mask = np.asarray(attention_mask, dtype=np.float32)[0, 0]
    Wq = np.asarray(Wq, dtype=np.float32)
    Wk = np.asarray(Wk, dtype=np.float32)
    Wv = np.asarray(Wv, dtype=np.float32)
    Wo = np.asarray(Wo, dtype=np.float32)

    classes = _mask_classes(mask)
    key = tuple(tuple(r) for r in classes)
    if key not in _CACHE:
        _CACHE[key] = _build(classes)
    nc = _CACHE[key]

    f8 = ml_dtypes.float8_e4m3fn
    wv16 = np.ascontiguousarray(Wv.astype(f16))
    wk16 = np.ascontiguousarray(Wk.astype(f16))
    wqg = np.ascontiguousarray(Wq[:, HS:].astype(f8))
    wo16 = np.ascontiguousarray(Wo.astype(f16))

    in_maps = []
    for core in range(NCORES):
        b, j = divmod(core, RANKS)
        blocks = [RANKS * k + j for k in range(SLOTS)]
        rows = np.concatenate([np.arange(bi * P, (bi + 1) * P) for bi in blocks])
        hidT = np.ascontiguousarray(hidden_states[b][rows].T.astype(f16))
        strips = []
        for k in range(SLOTS):
            for c in range(NCHUNK):
                if classes[k][c] == 2:
                    bi = RANKS * k + j
                    strips.append(mask[bi * P:(bi + 1) * P,
                                       c * CHUNK:(c + 1) * CHUNK])
        if not strips:
            strips.append(np.zeros((P, CHUNK), np.float32))
        in_maps.append({
            "hid16": hidT,
            "hid8": np.ascontiguousarray(hidT.astype(f8)),
            "wv16": wv16,
            "wk16": wk16,
            "wqg8": wqg,
            "wo": wo16,
            "cosT": np.ascontiguousarray(cos[b][rows].T),
            "sinT": np.ascontiguousarray(sin[b][rows].T),
            "maskst": np.ascontiguousarray(np.stack(strips).astype(bf)),
        })

    res = run_bass_kernel_spmd(nc, in_maps, core_ids=list(range(NCORES)))

    out = np.empty((B, S, HS), np.float32)
    for core in range(NCORES):
        b, j = divmod(core, RANKS)
        o = np.asarray(res.results[core]["out"]).astype(np.float32)
        for k in range(SLOTS):
            bi = RANKS * k + j
            out[b, bi * P:(bi + 1) * P, :] = o[k * P:(k + 1) * P, :]
    return out
